# revision 5
# baseline (speedup 1.0000x reference)
"""Trainium2 Bass kernel for nn_FHEBlock (dense transformer block, poly softmax).

Sharding: 8 cores = (batch 0..3) x (sequence half 0..1). Each core computes the
output rows for its (batch, half) slice [1024 tokens, 1024 features]. k/v are
computed per-core for the full 2048-token sequence of its batch (duplicated
across the half-pair) -- zero cross-core communication.

Layout: activations are kept feature-major ("xT" = [D, tokens]) so every matmul
uses a natural operand layout with no transposes:
  qT   = WQ(lhsT)  . xT(rhs)          kT = WK(lhsT) . xT(rhs)
  v    = xT(lhsT)  . WV(rhs)          (token-major)
  aT   = kT(lhsT)  . qT(rhs)          ([ktok, qtok])
  oT   = v(lhsT)   . poly(aT)(rhs)    attnT = WO(lhsT) . oT(rhs)
  h2T  = W1(lhsT)  . x1T(rhs)         outT = W2(lhsT) . poly(h2T)(rhs)

g1/g2 layernorm scales and all 0.1 factors are folded into the weights on the
host. Matmul inputs are bf16 (PSUM accumulation is fp32); the residual stream
stays fp32 end to end.
"""

import sys

for _p in ("/opt/trn_rl_repo",):
    if _p not in sys.path:
        sys.path.insert(0, _p)

import numpy as np
import ml_dtypes

import concourse.bass as bass
import concourse.mybir as mybir
import concourse.bacc as bacc
import concourse.tile as tile
from concourse.bass_utils import run_bass_kernel_spmd

P = 128
D = 1024
T = 2048          # kv tokens per core (full sequence of its batch)
Q = 1024          # q tokens per core (its half)
DI = D // P       # 8 feature chunks
TI = T // P       # 16 token chunks
NB = 512          # matmul moving free dim (one PSUM bank fp32)
BF = mybir.dt.bfloat16
F32 = mybir.dt.float32
AF = mybir.ActivationFunctionType

_CACHE = {}


def _build_program():
    nc = bacc.Bacc(target_bir_lowering=False)

    xbf = nc.declare_dram_parameter("xbf", [D, T], BF, isOutput=False)
    xa = nc.declare_dram_parameter("xa", [D, Q], F32, isOutput=False)
    wq = nc.declare_dram_parameter("wq", [D, D], BF, isOutput=False)
    wk = nc.declare_dram_parameter("wk", [D, D], BF, isOutput=False)
    wv = nc.declare_dram_parameter("wv", [D, D], BF, isOutput=False)
    wo = nc.declare_dram_parameter("wo", [D, D], BF, isOutput=False)
    w1 = nc.declare_dram_parameter("w1", [D, 2 * D], BF, isOutput=False)
    w2 = nc.declare_dram_parameter("w2", [2 * D, D], BF, isOutput=False)
    out = nc.declare_dram_parameter("out", [D, Q], F32, isOutput=True)

    r3 = lambda ap: ap.rearrange("(i p) f -> p i f", p=P)
    xbf3, xa3, wq3, wk3, wv3, wo3 = map(r3, (xbf, xa, wq, wk, wv, wo))
    w13, w23, out3 = map(r3, (w1, w2, out))

    with tile.TileContext(nc) as tc:
        # --- persistent pools (released last) ---
        psum = tc.alloc_tile_pool(name="psum", bufs=6, space="PSUM")
        tmp = tc.alloc_tile_pool(name="tmp", bufs=4)
        outp = tc.alloc_tile_pool(name="outp", bufs=3)
        cst = tc.alloc_tile_pool(name="cst", bufs=1)
        bias_half = cst.tile([P, 1], F32, name="bias_half")
        nc.any.memset(bias_half[:], 0.5)

        # --- left stack: wsm -> va -> qk (released qk, va, wsm), then mlp ---
        wsm = tc.alloc_tile_pool(name="wsm", bufs=18)
        va = tc.alloc_tile_pool(name="va", bufs=1)
        qk = tc.alloc_tile_pool(name="qk", bufs=1)
        # --- right stack: xb -> oxw -> w1p -> w2p (sequential) ---
        xbp = tc.alloc_tile_pool(name="xbp", bufs=1, side="right")

        v_s = va.tile([P, TI, D], BF)      # v token-major [tokP, tok chunk, D]
        a_s = va.tile([P, TI, Q], BF)      # poly(aT) [ktokP, ktok chunk, qtok]
        q_s = qk.tile([P, DI, Q], BF)      # qT feature-major
        k_s = qk.tile([P, DI, T], BF)      # kT feature-major
        xb_s = xbp.tile([P, DI, T], BF)    # xT bf16 (q half = cols 0:Q)

        for ki in range(DI):
            nc.sync.dma_start(out=xb_s[:, ki], in_=xbf3[:, ki])

        def load_w(src3, n):
            tiles = []
            for ki in range(n):
                w_t = wsm.tile([P, D], BF, tag="wch", name=f"wch{ki}")
                nc.sync.dma_start(out=w_t[:], in_=src3[:, ki])
                tiles.append(w_t)
            return tiles

        def mm_stage(n_m, n_n, n_k, lhsT_fn, rhs_fn, evict_fn):
            """for each m block: accumulate over k chunks into n_n interleaved
            PSUM banks (stationary operand reused across the n blocks)."""
            for mi in range(n_m):
                ps = [psum.tile([P, NB], F32, tag="ps", name=f"ps{mi}_{j}") for j in range(n_n)]
                for ki in range(n_k):
                    for nj in range(n_n):
                        nc.tensor.matmul(
                            ps[nj][:],
                            lhsT=lhsT_fn(ki, mi),
                            rhs=rhs_fn(ki, nj),
                            start=(ki == 0),
                            stop=(ki == n_k - 1),
                        )
                for nj in range(n_n):
                    evict_fn(mi, nj, ps[nj])

        def copy_evict(dst, chunks=1):
            def f(mi, nj, ps):
                nc.scalar.copy(dst[:, mi, nj * NB:(nj + 1) * NB], ps[:])
            return f

        def poly_evict(dst):
            # poly(a) = a^2 + a = (a + 0.5)^2 - 0.25 : Square on ScalarE,
            # then constant add + bf16 cast on VectorE.
            def f(mi, nj, ps):
                t = tmp.tile([P, NB], F32, tag="pt", name=f"pt{mi}_{nj}")
                nc.scalar.activation(t[:], ps[:], AF.Square, bias=bias_half[:])
                nc.vector.tensor_scalar_add(
                    dst[:, mi, nj * NB:(nj + 1) * NB], t[:], -0.25)
            return f

        # ---- phase 1: q, k, v projections ----
        wq_t = load_w(wq3, DI)
        mm_stage(DI, Q // NB, DI,
                 lambda ki, mi: wq_t[ki][:, mi * P:(mi + 1) * P],
                 lambda ki, nj: xb_s[:, ki, nj * NB:(nj + 1) * NB],
                 copy_evict(q_s))
        wk_t = load_w(wk3, DI)
        mm_stage(DI, T // NB, DI,
                 lambda ki, mi: wk_t[ki][:, mi * P:(mi + 1) * P],
                 lambda ki, nj: xb_s[:, ki, nj * NB:(nj + 1) * NB],
                 copy_evict(k_s))
        wv_t = load_w(wv3, DI)
        mm_stage(TI, D // NB, DI,
                 lambda ki, ti: xb_s[:, ki, ti * P:(ti + 1) * P],
                 lambda ki, nj: wv_t[ki][:, nj * NB:(nj + 1) * NB],
                 copy_evict(v_s))
        xbp.release()

        # ---- phase 2: aT = k @ qT, then poly ----
        mm_stage(TI, Q // NB, DI,
                 lambda ki, ti: k_s[:, ki, ti * P:(ti + 1) * P],
                 lambda ki, nj: q_s[:, ki, nj * NB:(nj + 1) * NB],
                 poly_evict(a_s))
        qk.release()

        oxw = tc.alloc_tile_pool(name="oxw", bufs=1, side="right")
        o_s = oxw.tile([P, DI, Q], BF)
        wo_s = oxw.tile([P, DI, D], BF)
        xa_s = oxw.tile([P, DI, Q], F32)
        for ki in range(DI):
            nc.sync.dma_start(out=wo_s[:, ki], in_=wo3[:, ki])
            nc.sync.dma_start(out=xa_s[:, ki], in_=xa3[:, ki])

        # ---- phase 3a: oT = vT . poly(aT)  (contract over 2048 kv tokens) ----
        mm_stage(DI, Q // NB, TI,
                 lambda ki, mi: v_s[:, ki, mi * P:(mi + 1) * P],
                 lambda ki, nj: a_s[:, ki, nj * NB:(nj + 1) * NB],
                 copy_evict(o_s))
        va.release()
        wsm.release()

        mlp = tc.alloc_tile_pool(name="mlp", bufs=1)
        x1f_s = mlp.tile([P, DI, Q], F32)
        x1b_s = mlp.tile([P, DI, Q], BF)
        h2_s = mlp.tile([P, TI, Q], BF)

        # ---- phase 3b: x1 = xa + WO . oT ----
        def x1_evict(mi, nj, ps):
            sl = (slice(None), mi, slice(nj * NB, (nj + 1) * NB))
            nc.vector.tensor_add(x1f_s[sl], ps[:], xa_s[sl])
            nc.scalar.copy(x1b_s[sl], x1f_s[sl])

        mm_stage(DI, Q // NB, DI,
                 lambda ki, mi: wo_s[:, ki, mi * P:(mi + 1) * P],
                 lambda ki, nj: o_s[:, ki, nj * NB:(nj + 1) * NB],
                 x1_evict)
        oxw.release()

        # ---- phase 4: h2 = poly(W1 . x1) ----
        w1p = tc.alloc_tile_pool(name="w1p", bufs=1, side="right")
        w1a = w1p.tile([P, DI, D], BF)
        w1b = w1p.tile([P, DI, D], BF)
        for ki in range(DI):
            nc.sync.dma_start(out=w1a[:, ki], in_=w13[:, ki, 0:D])
            nc.sync.dma_start(out=w1b[:, ki], in_=w13[:, ki, D:2 * D])

        def w1_lhsT(ki, mi):
            half, m = divmod(mi, DI)
            src = w1a if half == 0 else w1b
            return src[:, ki, m * P:(m + 1) * P]

        mm_stage(TI, Q // NB, DI, w1_lhsT,
                 lambda ki, nj: x1b_s[:, ki, nj * NB:(nj + 1) * NB],
                 poly_evict(h2_s))
        w1p.release()

        # ---- phase 5: out = x1 + W2 . h2 ----
        w2p = tc.alloc_tile_pool(name="w2p", bufs=1, side="right")
        w2_s = w2p.tile([P, TI, D], BF)
        for ki in range(TI):
            nc.sync.dma_start(out=w2_s[:, ki], in_=w23[:, ki])

        def out_evict(mi, nj, ps):
            sl = (slice(None), mi, slice(nj * NB, (nj + 1) * NB))
            ot = outp.tile([P, NB], F32, tag="ot", name=f"ot{mi}_{nj}")
            nc.vector.tensor_add(ot[:], ps[:], x1f_s[sl])
            nc.sync.dma_start(out=out3[sl], in_=ot[:])

        mm_stage(DI, Q // NB, TI,
                 lambda ki, mi: w2_s[:, ki, mi * P:(mi + 1) * P],
                 lambda ki, nj: h2_s[:, ki, nj * NB:(nj + 1) * NB],
                 out_evict)
        w2p.release()
        mlp.release()

        cst.release()
        outp.release()
        tmp.release()
        psum.release()

    nc.compile()
    return nc


def prep_inputs(x, Wq, Wk, Wv, Wo, W1, W2, g1, g2):
    """Host-side: fold scales into weights, shard, transpose to feature-major."""
    bf = ml_dtypes.bfloat16
    f32 = np.float32
    g1 = np.asarray(g1, f32)[:, None]
    g2 = np.asarray(g2, f32)[:, None]
    WQ = (0.01 * g1 * np.asarray(Wq, f32)).astype(bf)
    WK = (0.1 * g1 * np.asarray(Wk, f32)).astype(bf)
    WV = (g1 * np.asarray(Wv, f32)).astype(bf)
    WO = (0.1 * np.asarray(Wo, f32)).astype(bf)
    W1s = (g2 * np.asarray(W1, f32)).astype(bf)
    W2s = np.asarray(W2, f32).astype(bf)

    in_maps = []
    for c in range(8):
        b, h = divmod(c, 2)
        xt = np.ascontiguousarray(np.asarray(x[b], f32).T)  # [D, T]
        if h:
            xt = np.concatenate([xt[:, Q:], xt[:, :Q]], axis=1)
        in_maps.append({
            "xbf": xt.astype(bf),
            "xa": np.ascontiguousarray(xt[:, :Q]),
            "wq": WQ, "wk": WK, "wv": WV, "wo": WO, "w1": W1s, "w2": W2s,
        })
    return in_maps


def get_program():
    if "nc" not in _CACHE:
        _CACHE["nc"] = _build_program()
    return _CACHE["nc"]


def kernel(x, Wq, Wk, Wv, Wo, W1, W2, g1, g2):
    nc = get_program()
    in_maps = prep_inputs(x, Wq, Wk, Wv, Wo, W1, W2, g1, g2)
    res = run_bass_kernel_spmd(nc, in_maps, core_ids=list(range(8)))
    B = x.shape[0]
    out = np.empty((B, T, D), dtype=np.float32)
    for c in range(8):
        b, h = divmod(c, 2)
        out[b, h * Q:(h + 1) * Q, :] = res.results[c]["out"].T
    return out
